# revision 25
# baseline (speedup 1.0000x reference)
"""Trainium2 Bass kernel for nn_BinaryMasking (topk_masking).

Computes per-row top-K binary masks (src, tgt) over (B=128, N=65536) plus a
broadcast dR array, distributed over 8 NeuronCores (16 rows per core).

Method: rank order of F = log(u) + log(ue)/alpha equals rank order of the key
k = u * s with s = exp(log(ue)/alpha)  (monotone transform; s computed on host
from the tiny U_event input). The per-row top-K threshold is found on-device
with 4 fused count passes (tensor_scalar is_gt + accum_out) driven by a
safeguarded Newton iteration on the analytic CDF (inputs are iid uniform), then
an exact "snap" to the (K - cnt)-th largest element below the last threshold
using max8/match_replace. Final mask = (key >= theta) as uint8.
"""

import numpy as np

N = 65536
T = 16
HW = 4096          # h*w elements per t-block
B = 128
RPC = 16           # rows per core
FREE = 8192        # free dim per partition: row spread over 8 partitions
CLAMP = np.float32(1e-6)
NSIG = 8.0         # analytic bracket half-width in sigmas
SNAP_BIAS = 8.0    # undershoot bias on the last Newton step
AUXR_W = 76
AUXB_W = 8

_CACHE = {}
LAST_RESULTS = None


# --------------------------------------------------------------------------- #
# Host-side prep (uses only the small inputs U_event/U_rate + the spec'd
# uniform distribution of U_weight; all heavy data is processed on device).
# --------------------------------------------------------------------------- #

def _host_prep(U_event, U_rate):
    ue = np.clip(U_event.astype(np.float32), CLAMP, np.float32(1.0) - CLAMP)
    s_src = np.sqrt(ue, dtype=np.float32)                       # (B, T)
    s_tgt = np.sqrt((np.float32(1.0) - ue), dtype=np.float32)   # (B, T)
    L = np.linspace(0.01, 0.99, B, dtype=np.float32)
    r = np.clip(U_rate.astype(np.float32).ravel()[0], CLAMP, np.float32(1.0) - CLAMP)
    ssum = (L + r).astype(np.float32)
    # jax f32 `% 1.0` lowers to IEEE-remainder style x - rint(x): U can be < 0!
    U = (ssum - np.rint(ssum)).astype(np.float32)
    with np.errstate(invalid="ignore", divide="ignore"):
        R_src = np.exp(np.log(U) / np.float32(3.0))      # NaN for U <= 0
        R_tgt = np.exp(np.log(np.float32(1.0) - U))      # exp(log(1-U)/1)
        K_src = np.where(np.isnan(R_src), 0.0, R_src * N)
        K_tgt = np.where(np.isnan(R_tgt), 0.0, R_tgt * N)
        K_src = np.clip(K_src, 0, N).astype(np.int32)
        K_tgt = np.clip(K_tgt, 0, N).astype(np.int32)
        dR = (np.exp(np.log(U) * np.float32(1.0 / 3.0 - 1.0)) /
              np.float32(3.0)).astype(np.float32)
    return s_src, s_tgt, K_src, K_tgt, dR


def _analytic_init(s, K, nsig=NSIG):
    """Per-row analytic threshold t0 and bracket [lo, hi] from the uniform-key
    CDF: E[cnt_gt(x)] = HW * sum_t max(0, 1 - x/s_t)."""
    Bn = s.shape[0]
    t0 = np.zeros(Bn); lo = np.zeros(Bn); hi = np.zeros(Bn)

    def inv_cnt(svals, C):
        C = min(max(C, 0.0), float(T * HW))
        knots = np.concatenate([np.sort(svals)[::-1], [0.0]])
        Ek = np.array([HW * np.sum(np.maximum(0.0, 1.0 - k / svals))
                       for k in knots])
        i = int(np.searchsorted(Ek, C)) - 1
        i = min(max(i, 0), len(knots) - 2)
        x1, x2, E1, E2 = knots[i], knots[i + 1], Ek[i], Ek[i + 1]
        if E2 == E1:
            return float(x1)
        return float(x1 + (C - E1) * (x2 - x1) / (E2 - E1))

    for b in range(Bn):
        sv = s[b].astype(np.float64)
        Kb = float(K[b])
        if Kb <= 0:
            t0[b] = lo[b] = hi[b] = 2.0
            continue
        if Kb >= N:
            t0[b] = lo[b] = hi[b] = 0.0
            continue
        x0 = inv_cnt(sv, Kb)
        p = np.maximum(0.0, 1.0 - x0 / sv)
        sig = float(np.sqrt(np.sum(HW * p * (1 - p)) + 1.0))
        lo[b] = inv_cnt(sv, Kb + nsig * sig)
        hi[b] = inv_cnt(sv, Kb - nsig * sig)
        t0[b] = x0
    # analytic |dE/dx| at t0 (stale across rounds; fine for Newton)
    dens = (HW / s.astype(np.float64) * (t0[:, None] < s)).sum(1)
    invd = 1.0 / (dens + 1e-3)
    return (t0.astype(np.float32), lo.astype(np.float32), hi.astype(np.float32),
            invd.astype(np.float32))


def _band(x16):
    """(16,) row values -> (128,) banded over partitions (row r on 8r..8r+7)."""
    return np.repeat(np.asarray(x16, np.float32), 8)


def _make_inputs(U_weight, U_event, U_rate):
    s_src, s_tgt, K_src, K_tgt, dR = _host_prep(U_event, U_rate)
    t0s, los, his, ids_ = _analytic_init(s_src, K_src)
    t0t, lot, hit, idt_ = _analytic_init(s_tgt, K_tgt)

    bmat3 = np.zeros((128, 16), np.float32)      # row-sum reduce: [128,1]->[16,1]
    for k in range(128):
        bmat3[k, k // 8] = 1.0
    bmat2 = np.zeros((16, 128), np.float32)      # row -> banded broadcast
    for p in range(128):
        bmat2[p // 8, p] = 1.0
    iota16 = np.broadcast_to(np.arange(16, dtype=np.float32), (16, 16)).copy()

    in_maps = []
    for c in range(8):
        rows = slice(16 * c, 16 * c + 16)
        uw = np.ascontiguousarray(
            U_weight[:, rows, :].reshape(2, 16, 8, FREE).reshape(256, FREE))

        auxb = np.zeros((128, AUXB_W), np.float32)
        j = np.arange(128) % 8
        rr = np.arange(128) // 8 + 16 * c
        auxb[:, 0] = s_src[rr, 2 * j]
        auxb[:, 1] = s_src[rr, 2 * j + 1]
        auxb[:, 2] = s_tgt[rr, 2 * j]
        auxb[:, 3] = s_tgt[rr, 2 * j + 1]
        auxb[:, 4] = _band(t0s[rows])
        auxb[:, 5] = _band(t0t[rows])
        auxb[:, 6] = -_band(t0t[rows])
        auxb[:, 7] = _band(dR[rows])

        # sign-count: tgt row count = (S + 8*FREE)/2 = 0.5*S + N/2
        Ct = np.float32(N // 2) - K_tgt[rows].astype(np.float32)
        Ks = K_src[rows].astype(np.float32)
        ivs = ids_[rows]
        ivt = idt_[rows]
        auxr = np.zeros((16, AUXR_W), np.float32)
        auxr[:, 3] = t0s[rows]
        auxr[:, 4] = t0t[rows]
        auxr[:, 5] = los[rows]
        auxr[:, 6] = lot[rows]
        auxr[:, 7] = his[rows]
        auxr[:, 8] = hit[rows]
        auxr[:, 9] = ivs                      # a2: t' = t + c*a2 + b2
        auxr[:, 10] = np.float32(0.5) * ivt
        auxr[:, 11] = -Ks * ivs               # b2
        auxr[:, 12] = Ct * ivt
        auxr[:, 13] = np.float32(SNAP_BIAS) * ivs   # biasoff2
        auxr[:, 14] = np.float32(SNAP_BIAS) * ivt
        auxr[:, 15] = np.float32(-1.0)        # nalpha2 (m'' = c*na + nb)
        auxr[:, 16] = np.float32(-0.5)
        auxr[:, 17] = Ks                      # nbeta2
        auxr[:, 18] = -Ct

        in_maps.append({
            "uw": uw, "auxb": auxb, "auxr": auxr,
            "bmat3": bmat3, "bmat2": bmat2, "iota16": iota16,
        })
    return in_maps, dR


# --------------------------------------------------------------------------- #
# Bass kernel
# --------------------------------------------------------------------------- #

def _build_bass(NR=3, do_snap=True, do_dr=True, do_keys=True):
    import concourse.mybir as mybir
    from concourse.bacc import Bacc
    from concourse.tile import TileContext

    f32 = mybir.dt.float32
    u8 = mybir.dt.uint8
    op = mybir.AluOpType

    nc = Bacc("TRN2", target_bir_lowering=False)
    uw = nc.dram_tensor("uw", [256, FREE], f32, kind="ExternalInput")
    auxb = nc.dram_tensor("auxb", [128, AUXB_W], f32, kind="ExternalInput")
    auxr = nc.dram_tensor("auxr", [16, AUXR_W], f32, kind="ExternalInput")
    bmat3 = nc.dram_tensor("bmat3", [128, 16], f32, kind="ExternalInput")
    bmat2 = nc.dram_tensor("bmat2", [16, 128], f32, kind="ExternalInput")
    iota16 = nc.dram_tensor("iota16", [16, 16], f32, kind="ExternalInput")
    msrc = nc.dram_tensor("msrc", [128, FREE], u8, kind="ExternalOutput")
    mtgt = nc.dram_tensor("mtgt", [128, FREE], u8, kind="ExternalOutput")
    dro = nc.dram_tensor("dro", [128, FREE], f32, kind="ExternalOutput")

    with TileContext(nc) as tc:
        with (
            tc.tile_pool(name="big", bufs=1) as big,
            tc.tile_pool(name="small", bufs=2) as small,
            tc.tile_pool(name="psum", bufs=1, space="PSUM") as psum,
        ):
            # ---- small constants in ----
            auxb_t = small.tile([128, AUXB_W], f32, tag="auxb")
            auxr_t = small.tile([16, AUXR_W], f32, tag="auxr")
            bmat3_t = small.tile([128, 16], f32, tag="bmat3")
            bmat2_t = small.tile([16, 128], f32, tag="bmat2")
            iota_t = small.tile([16, 16], f32, tag="iota16")
            nc.sync.dma_start(out=auxb_t, in_=auxb[:, :])
            nc.sync.dma_start(out=auxr_t, in_=auxr[:, :])
            nc.sync.dma_start(out=bmat3_t, in_=bmat3[:, :])
            nc.sync.dma_start(out=bmat2_t, in_=bmat2[:, :])
            nc.sync.dma_start(out=iota_t, in_=iota16[:, :])

            # ---- big tiles ----
            key = {}
            mask_t = {}
            for mi, mname in ((0, "src"), (1, "tgt")):
                kt = big.tile([128, FREE], f32, tag=f"key{mi}")
                for ch in range(4):
                    sl = slice(ch * 2048, (ch + 1) * 2048)
                    nc.sync.dma_start(out=kt[:, sl],
                                      in_=uw[128 * mi:128 * (mi + 1), sl])
                key[mi] = kt
                mask_t[mi] = big.tile([128, FREE], u8, tag=f"mask{mi}",
                                      name=f"mask{mi}")
            mb_t = big.tile([128, FREE], f32, tag="mb")      # masked-below (src)
            jt_t = big.tile([128, FREE], f32, tag="jt")      # tgt ACT junk + mb tgt
            dr_t = big.tile([128, FREE], f32, tag="dr")

            # ---- key build: key = max(u, 1e-6) * s_half  (in place) ----
            # chunked to match the DMA chunks; tensor_scalar w/ per-partition s
            for mi in (0, 1) if do_keys else ():
                for ch in range(4):
                    sl = slice(ch * 2048, (ch + 1) * 2048)
                    h = ch // 2
                    nc.vector.tensor_scalar(
                        out=key[mi][:, sl], in0=key[mi][:, sl],
                        scalar1=float(CLAMP),
                        scalar2=auxb_t[:, 2 * mi + h:2 * mi + h + 1],
                        op0=op.max, op1=op.mult)

            # ---- dR broadcast on ACT: Identity(0*x + dR), NaN-propagating ----
            if do_dr:
                nc.scalar.activation(
                    out=dr_t, in_=key[0],
                    func=mybir.ActivationFunctionType.Identity,
                    bias=auxb_t[:, 7:8], scale=0.0)
                for ch in range(4):
                    sl = slice(ch * 2048, (ch + 1) * 2048)
                    nc.sync.dma_start(out=dro[:, sl], in_=dr_t[:, sl])

            # ---- Newton rounds (batched across the two masks) ----
            # auxr columns (see _make_inputs): 3:5 t0, 5:7 lo, 7:9 hi,
            # 9:11 a2, 11:13 b2, 13:15 biasoff2, 15:17 nalpha2, 17:19 nbeta2
            t2_cur = auxr_t[:, 3:5]                  # [16,2] (src, tgt)
            tb_s = auxb_t[:, 4:5]                    # banded +t (src)
            tb_tn = auxb_t[:, 6:7]                   # banded -t (tgt)
            tb_tp = None                             # banded +t3 (tgt), for mb
            c2_last = None
            for rd in range(NR):
                cnt2 = small.tile([128, 2], f32, tag="cnt2", name=f"cnt2_{rd}")
                nc.vector.tensor_scalar(
                    out=mask_t[0], in0=key[0], scalar1=tb_s,
                    scalar2=None, op0=op.is_gt, op1=op.add,
                    accum_out=cnt2[:, 0:1])
                nc.scalar.activation(
                    out=jt_t, in_=key[1],
                    func=mybir.ActivationFunctionType.Sign,
                    bias=tb_tn, scale=1.0, accum_out=cnt2[:, 1:2])
                c2 = psum.tile([16, 2], f32, tag="c2", name=f"c2_{rd}")
                nc.tensor.matmul(c2, bmat3_t, cnt2, start=True, stop=True)
                c2_last = c2
                if rd == NR - 1:
                    break
                # t' = clamp(t + c*a2 + b2 (+ biasoff2 on the last step))
                v1 = small.tile([16, 2], f32, tag="v1", name=f"v1_{rd}")
                nc.vector.tensor_tensor(out=v1, in0=c2, in1=auxr_t[:, 9:11],
                                        op=op.mult)
                v2 = small.tile([16, 2], f32, tag="v2", name=f"v2_{rd}")
                nc.vector.tensor_tensor(out=v2, in0=v1, in1=auxr_t[:, 11:13],
                                        op=op.add)
                if rd == NR - 2:
                    v2b = small.tile([16, 2], f32, tag="v2b", name=f"v2b_{rd}")
                    nc.vector.tensor_tensor(out=v2b, in0=v2,
                                            in1=auxr_t[:, 13:15], op=op.add)
                    v2 = v2b
                v3 = small.tile([16, 2], f32, tag="v3", name=f"v3_{rd}")
                nc.vector.tensor_tensor(out=v3, in0=v2, in1=t2_cur, op=op.add)
                v4 = small.tile([16, 2], f32, tag="v4", name=f"v4_{rd}")
                nc.vector.tensor_tensor(out=v4, in0=v3, in1=auxr_t[:, 5:7],
                                        op=op.max)
                t2n = small.tile([16, 2], f32, tag="t2n", name=f"t2n_{rd}")
                nc.vector.tensor_tensor(out=t2n, in0=v4, in1=auxr_t[:, 7:9],
                                        op=op.min)
                t2_cur = t2n
                tbp2 = psum.tile([128, 2], f32, tag="tbp2", name=f"tbp2_{rd}")
                nc.tensor.matmul(tbp2, bmat2_t, t2n, start=True, stop=True)
                tbs = small.tile([128, 1], f32, tag="tbs", name=f"tbs_{rd}")
                nc.vector.tensor_copy(out=tbs, in_=tbp2[:, 0:1])
                tb_s = tbs
                tbn = small.tile([128, 1], f32, tag="tbn", name=f"tbn_{rd}")
                nc.vector.tensor_scalar(out=tbn, in0=tbp2[:, 1:2],
                                        scalar1=-1.0, scalar2=None, op0=op.mult)
                tb_tn = tbn
                if rd == NR - 2:
                    tbtp = small.tile([128, 1], f32, tag="tbtp", name="tbtp")
                    nc.vector.tensor_copy(out=tbtp, in_=tbp2[:, 1:2])
                    tb_tp = tbtp

            # ---- snap: theta = (K - c3)-th largest strictly below t3 ----
            # masked-below tiles: src on GPSIMD (overlaps round-3 counts),
            # tgt on DVE.
            if do_snap:
                nc.vector.scalar_tensor_tensor(
                    out=mb_t, in0=key[0], scalar=tb_s, in1=key[0],
                    op0=op.is_lt, op1=op.mult)
                nc.vector.scalar_tensor_tensor(
                    out=jt_t, in0=key[1], scalar=tb_tp, in1=key[1],
                    op0=op.is_lt, op1=op.mult)
                top16 = {}
                for mi, mb in ((0, mb_t), (1, jt_t)):
                    pmax = small.tile([128, 8], f32, tag=f"pmax{mi}",
                                      name=f"pmax{mi}")
                    nc.vector.max(out=pmax, in_=mb)
                    rowbuf = small.tile([16, 64], f32, tag=f"rowbuf{mi}",
                                        name=f"rowbuf{mi}")
                    nc.sync.dma_start(out=rowbuf, in_=pmax)
                    t16 = small.tile([16, 16], f32, tag=f"top16_{mi}",
                                     name=f"top16_{mi}")
                    nc.vector.max(out=t16[:, 0:8], in_=rowbuf)
                    rowzap = small.tile([16, 64], f32, tag=f"rowzap{mi}",
                                        name=f"rowzap{mi}")
                    nc.vector.match_replace(
                        out=rowzap, in_to_replace=t16[:, 0:8],
                        in_values=rowbuf, imm_value=-1.0)
                    nc.vector.max(out=t16[:, 8:16], in_=rowzap)
                    top16[mi] = t16

                # m'' = K - c3 = c2*nalpha2 + nbeta2 ; sel = (m'' <= 0)
                w1 = small.tile([16, 2], f32, tag="w1", name="w1")
                nc.vector.tensor_tensor(out=w1, in0=c2_last,
                                        in1=auxr_t[:, 15:17], op=op.mult)
                mraw2 = small.tile([16, 2], f32, tag="mraw2", name="mraw2")
                nc.vector.tensor_tensor(out=mraw2, in0=w1,
                                        in1=auxr_t[:, 17:19], op=op.add)
                sel2 = small.tile([16, 2], f32, tag="sel2", name="sel2")
                nc.vector.tensor_scalar(out=sel2, in0=mraw2, scalar1=0.5,
                                        scalar2=None, op0=op.is_lt)
                mcl2 = small.tile([16, 2], f32, tag="mcl2", name="mcl2")
                nc.vector.tensor_scalar(out=mcl2, in0=mraw2, scalar1=1.0,
                                        scalar2=16.0, op0=op.max, op1=op.min)
                mm12 = small.tile([16, 2], f32, tag="mm12", name="mm12")
                nc.vector.tensor_scalar(out=mm12, in0=mcl2, scalar1=1.0,
                                        scalar2=None, op0=op.subtract)
                thsnap2 = small.tile([16, 2], f32, tag="thsnap2", name="thsnap2")
                for mi in (0, 1):
                    oh = small.tile([16, 16], f32, tag=f"oh{mi}", name=f"oh{mi}")
                    nc.vector.tensor_scalar(
                        out=oh, in0=iota_t, scalar1=mm12[:, mi:mi + 1],
                        scalar2=None, op0=op.is_equal)
                    ohj = small.tile([16, 16], f32, tag=f"ohj{mi}",
                                     name=f"ohj{mi}")
                    nc.vector.scalar_tensor_tensor(
                        out=ohj, in0=oh, scalar=1.0, in1=top16[mi],
                        op0=op.mult, op1=op.mult,
                        accum_out=thsnap2[:, mi:mi + 1])
                # theta = thsnap + sel*(t3 - thsnap)
                dd2 = small.tile([16, 2], f32, tag="dd2", name="dd2")
                nc.vector.tensor_tensor(out=dd2, in0=t2_cur, in1=thsnap2,
                                        op=op.subtract)
                sd2 = small.tile([16, 2], f32, tag="sd2", name="sd2")
                nc.vector.tensor_mul(out=sd2, in0=sel2, in1=dd2)
                th2 = small.tile([16, 2], f32, tag="th2", name="th2")
                nc.vector.tensor_add(out=th2, in0=thsnap2, in1=sd2)
            else:
                th2 = t2_cur
            thp2 = psum.tile([128, 2], f32, tag="thp2", name="thp2")
            nc.tensor.matmul(thp2, bmat2_t, th2, start=True, stop=True)
            thb2 = small.tile([128, 2], f32, tag="thb2", name="thb2")
            nc.vector.tensor_copy(out=thb2, in_=thp2)

            # ---- final masks (chunked; DMA overlaps compute) ----
            for mi, out_dram in ((0, msrc), (1, mtgt)):
                for ch in range(2):
                    sl = slice(ch * 4096, (ch + 1) * 4096)
                    nc.vector.tensor_scalar(
                        out=mask_t[mi][:, sl], in0=key[mi][:, sl],
                        scalar1=thb2[:, mi:mi + 1], scalar2=None, op0=op.is_ge)
                    nc.sync.dma_start(out=out_dram[:, sl],
                                      in_=mask_t[mi][:, sl])
    nc.compile()
    return nc


# --------------------------------------------------------------------------- #
# Entry point
# --------------------------------------------------------------------------- #

def kernel(U_weight, U_event, U_rate, B=128):
    global LAST_RESULTS
    U_weight = np.asarray(U_weight, dtype=np.float32)
    U_event = np.asarray(U_event, dtype=np.float32)
    U_rate = np.asarray(U_rate, dtype=np.float32)
    assert U_weight.shape == (2, 128, N), U_weight.shape

    in_maps, dR = _make_inputs(U_weight, U_event, U_rate)

    if "nc" not in _CACHE:
        _CACHE["nc"] = _build_bass()
    nc = _CACHE["nc"]

    from concourse.bass_utils import run_bass_kernel_spmd
    res = run_bass_kernel_spmd(nc, in_maps, core_ids=list(range(8)))
    LAST_RESULTS = res

    src = np.empty((128, N), dtype=bool)
    tgt = np.empty((128, N), dtype=bool)
    dro = np.empty((128, N), dtype=np.float32)
    for c in range(8):
        rows = slice(16 * c, 16 * c + 16)
        out = res.results[c]
        src[rows] = out["msrc"].reshape(16, 8 * FREE) != 0
        tgt[rows] = out["mtgt"].reshape(16, 8 * FREE) != 0
        dro[rows] = out["dro"].reshape(16, 8 * FREE)
    return src, tgt, dro
